# revision 20
# baseline (speedup 1.0000x reference)
"""Trainium2 kernel for 4096x4096 single-channel 7x7 valid cross-correlation + bias.

Strategy (v10): 4x concurrent 64x64 PE tiles, wide DMAs, PE-bound pipeline
--------------------------------------------------------------------------
HW model (probe-measured on this part):
- Tiled matmul throughput is bound by the serialized per-matmul LDWEIGHTS
  stream + semaphore update: ~(stationary_cols/1.2GHz + ~12ns + sem) per
  matmul; 64-col loads measure ~60-64 ns/MM.  Output rows per LDW-time is
  maximized at M=58 (64x64 tiles): 0.97 rows/ns vs 0.65 at M=26.
- DMA queues run ~3x faster on 128-partition-wide transfers than 32-wide
  ones, so the layout avoids on-chip shifted copies entirely (they would
  need narrow partition-interleaved writes).

Decomposition: y[r,c] = sum_j sum_i W[i,j] x[r+i, c+j] as 7 banded-Toeplitz
matmuls (one per kernel column j) accumulated in PSUM:
- Strips of 64 input rows -> 58 output rows; contraction K=64 (strip rows),
  lhsT [64, 58] = T_j (band W[u-m, j]); moving = strip block shifted j cols
  in the free dim (a slice, not a copy); N=512 output cols.
- 4 concurrent 64x64 PE tiles (r2, c2): tile streams SBUF partitions 64*r2,
  accumulates into PSUM bank r2 partitions [64*c2, 64*c2+58); strip
  4R + 2*r2 + c2 lives in partition half r2, free slot c2.
- Round = 4 strips: one full-width input DMA [128, 2080B lines] (sync),
  28 matmuls, 2 drains [128, 512] f32->bf16+bias (ScalarE/VectorE), and per
  round-pair 2 stores [58, 4096B lines] (gpsimd).  72 strip slots = 18
  rounds; PSUM 2 banks/round, 4-deep double buffering.

Sharding: output columns across 8 cores (512 each + 6-col halo host-side).
"""

import os

import numpy as np
import ml_dtypes

import concourse.bass as bass
import concourse.bacc as bacc_mod
import concourse.mybir as mybir
import concourse.tile as tile
from concourse.bass_utils import run_bass_kernel_spmd

H = 4096          # input rows
W = 4096          # input cols
KH = 7            # kernel rows
KW = 7            # kernel cols
OH = H - KH + 1   # 4090 output rows
OW = W - KW + 1   # 4090 output cols
NCORES = 8
CW = 512          # output cols per core
SW = CW + KW - 1  # 518 input cols per shard

TS = 64           # input rows per strip
SOUT = TS - KH + 1  # 58 output rows per strip
NROUNDS = 18      # 4 strips per round
NSTRIPS = 4 * NROUNDS           # 72 strip slots (71 real)
FREEW = 1040                    # 2 slots x 518 + pad (2080B lines)
PAD_ROWS = SOUT * (NSTRIPS - 1) + TS  # 4182

_BF16 = ml_dtypes.bfloat16


def _build_program(bias_val: float) -> bass.Bass:
    nc = bacc_mod.Bacc("TRN2", target_bir_lowering=False)

    x_d = nc.dram_tensor("xs", [NROUNDS, 128, FREEW], mybir.dt.bfloat16,
                         kind="ExternalInput")
    w_d = nc.dram_tensor("tmat", [128, KW * SOUT], mybir.dt.bfloat16,
                         kind="ExternalInput")
    # y[P, c2, q, (rp, r2, w) flat]: strip 8P + 4rp + 2r2 + c2, output row
    # q.  The last dim stays flat so each store line is ONE contiguous
    # 4KB DMA descriptor (nested dims fragment into 1KB packets, 72GB/s).
    y_d = nc.dram_tensor("y", [NROUNDS // 2, 2, SOUT, 4 * CW],
                         mybir.dt.bfloat16, kind="ExternalOutput")

    with tile.TileContext(nc) as tc:
        with (
            tc.tile_pool(name="const", bufs=1) as constp,
            tc.tile_pool(name="xg", bufs=4) as xgp,
            tc.tile_pool(name="yg", bufs=2) as ygp,
            tc.tile_pool(name="ps", bufs=8, space="PSUM") as psp,
        ):
            w_sb = constp.tile([128, KW * SOUT], mybir.dt.bfloat16)
            nc.sync.dma_start(w_sb[:, :], w_d[:, :])

            xg_tiles = {}

            def load_round(R):
                xg = xgp.tile([128, FREEW], mybir.dt.bfloat16,
                              name="xg", tag="xg")
                xg_tiles[R] = xg
                nc.sync.dma_start(xg[:, :], x_d[R])

            load_round(0)
            load_round(1)

            yt_pair = None
            for R in range(NROUNDS):
                if R + 2 < NROUNDS:
                    load_round(R + 2)
                xg = xg_tiles.pop(R)
                ps_tiles = [psp.tile([128, CW], mybir.dt.float32,
                                     name="ps", tag="ps")
                            for _ in range(2)]

                for j in range(KW):
                    for r2 in range(2):
                        for c2 in range(2):
                            off = c2 * SW + j
                            nc.tensor.matmul(
                                ps_tiles[r2][64 * c2:64 * c2 + SOUT, :],
                                w_sb[64 * r2:64 * r2 + 64,
                                     SOUT * j:SOUT * j + SOUT],
                                xg[64 * r2:64 * r2 + 64, off:off + CW],
                                start=(j == 0),
                                stop=(j == KW - 1),
                                tile_position=(64 * r2, 64 * c2),
                            )

                rp = R % 2
                if rp == 0:
                    yt_pair = ygp.tile([128, 4 * CW], mybir.dt.bfloat16,
                                       name="yg", tag="yg")
                for r2 in range(2):
                    dst = yt_pair[:, (2 * rp + r2) * CW:(2 * rp + r2 + 1) * CW]
                    src = ps_tiles[r2][:, :]
                    if r2 == 0:
                        nc.scalar.activation(
                            dst, src, mybir.ActivationFunctionType.Copy,
                            bias=float(bias_val),
                        )
                    else:
                        nc.vector.tensor_scalar_add(dst, src, float(bias_val))

                if rp == 1:
                    # SBUF->DRAM runs ~100 GB/s per queue; rotate stores
                    # across all three DMA-capable engines.
                    for c2 in range(2):
                        eng = (nc.gpsimd, nc.scalar, nc.sync)[(R + c2) % 3]
                        eng.dma_start(
                            y_d[R // 2, c2],
                            yt_pair[64 * c2:64 * c2 + SOUT, :],
                        )

    nc.compile()
    nc.finalize()
    return nc


def _toeplitz(weight: np.ndarray) -> np.ndarray:
    """[128, 7*58] bf16: block j holds T_j[u, m] = W[u-m, j] (band 0<=u-m<7),
    u in [0,64), m in [0,58); replicated for partition half r2=1."""
    t = np.zeros((TS, KW * SOUT), np.float32)
    for j in range(KW):
        for i in range(KH):
            mm = np.arange(0, SOUT)
            t[mm + i, j * SOUT + mm] = weight[i, j]
    return np.tile(t, (2, 1)).astype(_BF16)


def _pack_shard(x_bf: np.ndarray, c0: int) -> np.ndarray:
    """[18, 128, 1040] bf16: partition 64*r2 + p of round R, free slot c2
    holds row 58*(4R + 2*r2 + c2) + p."""
    valid = min(SW, W - c0)
    xs = np.zeros((PAD_ROWS, SW), _BF16)
    xs[:H, :valid] = x_bf[:, c0:c0 + valid]
    R = np.arange(NROUNDS)
    out = np.zeros((NROUNDS, 128, FREEW), _BF16)
    for r2 in range(2):
        for c2 in range(2):
            s = 4 * R + 2 * r2 + c2
            rows = SOUT * s[:, None] + np.arange(TS)[None, :]
            out[:, 64 * r2:64 * r2 + TS, c2 * SW:(c2 + 1) * SW] = xs[rows]
    return out


def _unpack_out(y_packed: np.ndarray) -> np.ndarray:
    """[9, 2, 58, 2, 2, 512] bf16 -> [4090, 512] f32 (strip 8P+4rp+2r2+c2)."""
    y = y_packed.reshape(NROUNDS // 2, 2, SOUT, 2, 2, CW)
    y = y.transpose(0, 3, 4, 1, 2, 5)   # [P, rp, r2, c2, q, w]
    return y.reshape(NSTRIPS * SOUT, CW)[:OH].astype(np.float32)


def kernel(x: np.ndarray, weight: np.ndarray, bias: np.ndarray) -> np.ndarray:
    x = np.asarray(x, dtype=np.float32)
    weight = np.asarray(weight, dtype=np.float32)
    bias = np.asarray(bias, dtype=np.float32)

    tmat = _toeplitz(weight)
    x_bf = x.astype(_BF16)

    in_maps = []
    for c in range(NCORES):
        in_maps.append({"xs": _pack_shard(x_bf, CW * c), "tmat": tmat})

    nc = _build_program(float(bias[0]))

    trace = bool(int(os.environ.get("CONV_KERNEL_TRACE", "0")))
    res = run_bass_kernel_spmd(nc, in_maps, core_ids=list(range(NCORES)),
                               trace=trace)
    if trace:
        kernel.last_exec_time_ns = res.exec_time_ns

    cols = []
    for c in range(NCORES):
        valid_out = min(CW, OW - CW * c)
        cols.append(_unpack_out(np.asarray(res.results[c]["y"]))[:, :valid_out])
    return np.concatenate(cols, axis=1).astype(np.float32)


# revision 21
# speedup vs baseline: 1.4680x; 1.4680x over previous
"""Trainium2 kernel for 4096x4096 single-channel 7x7 valid cross-correlation + bias.

Strategy (v10): 4x concurrent 64x64 PE tiles, wide DMAs, PE-bound pipeline
--------------------------------------------------------------------------
HW model (probe-measured on this part):
- Tiled matmul throughput is bound by the serialized per-matmul LDWEIGHTS
  stream + semaphore update: ~(stationary_cols/1.2GHz + ~12ns + sem) per
  matmul; 64-col loads measure ~60-64 ns/MM.  Output rows per LDW-time is
  maximized at M=58 (64x64 tiles): 0.97 rows/ns vs 0.65 at M=26.
- DMA queues run ~3x faster on 128-partition-wide transfers than 32-wide
  ones, so the layout avoids on-chip shifted copies entirely (they would
  need narrow partition-interleaved writes).

Decomposition: y[r,c] = sum_j sum_i W[i,j] x[r+i, c+j] as 7 banded-Toeplitz
matmuls (one per kernel column j) accumulated in PSUM:
- Strips of 64 input rows -> 58 output rows; contraction K=64 (strip rows),
  lhsT [64, 58] = T_j (band W[u-m, j]); moving = strip block shifted j cols
  in the free dim (a slice, not a copy); N=512 output cols.
- 4 concurrent 64x64 PE tiles (r2, c2): tile streams SBUF partitions 64*r2,
  accumulates into PSUM bank r2 partitions [64*c2, 64*c2+58); strip
  4R + 2*r2 + c2 lives in partition half r2, free slot c2.
- Round = 4 strips: one full-width input DMA [128, 2080B lines] (sync),
  28 matmuls, 2 drains [128, 512] f32->bf16+bias (ScalarE/VectorE), and per
  round-pair 2 stores [58, 4096B lines] (gpsimd).  72 strip slots = 18
  rounds; PSUM 2 banks/round, 4-deep double buffering.

Sharding: output columns across 8 cores (512 each + 6-col halo host-side).
"""

import os

import numpy as np
import ml_dtypes

import concourse.bass as bass
import concourse.bacc as bacc_mod
import concourse.mybir as mybir
import concourse.tile as tile
from concourse.bass_utils import run_bass_kernel_spmd

H = 4096          # input rows
W = 4096          # input cols
KH = 7            # kernel rows
KW = 7            # kernel cols
OH = H - KH + 1   # 4090 output rows
OW = W - KW + 1   # 4090 output cols
NCORES = 8
CW = 512          # output cols per core
SW = CW + KW - 1  # 518 input cols per shard

TS = 64           # input rows per strip
SOUT = TS - KH + 1  # 58 output rows per strip
NROUNDS = 18      # 4 strips per round
NSTRIPS = 4 * NROUNDS           # 72 strip slots (71 real)
FREEW = 1040                    # 2 slots x 518 + pad (2080B lines)
PAD_ROWS = SOUT * (NSTRIPS - 1) + TS  # 4182

_BF16 = ml_dtypes.bfloat16


def _build_program(bias_val: float) -> bass.Bass:
    nc = bacc_mod.Bacc("TRN2", target_bir_lowering=False)

    x_d = nc.dram_tensor("xs", [NROUNDS, 128, FREEW], mybir.dt.bfloat16,
                         kind="ExternalInput")
    w_d = nc.dram_tensor("tmat", [128, KW * SOUT], mybir.dt.bfloat16,
                         kind="ExternalInput")
    # y[c2, q, u*512 + w] with u = 2R + r2: strip 4R + 2r2 + c2, row q.
    # Lines stay flat/contiguous and stores pack 4 rounds -> 16KB
    # descriptors (store queues pace ~42ns/packet regardless of size).
    y_d = nc.dram_tensor("y", [2, SOUT, 2 * NROUNDS * CW],
                         mybir.dt.bfloat16, kind="ExternalOutput")

    with tile.TileContext(nc) as tc:
        with (
            tc.tile_pool(name="const", bufs=1) as constp,
            tc.tile_pool(name="xg", bufs=4) as xgp,
            tc.tile_pool(name="yg", bufs=2) as ygp,
            tc.tile_pool(name="ps", bufs=8, space="PSUM") as psp,
        ):
            w_sb = constp.tile([128, KW * SOUT], mybir.dt.bfloat16)
            nc.sync.dma_start(w_sb[:, :], w_d[:, :])

            xg_tiles = {}

            def load_round(R):
                xg = xgp.tile([128, FREEW], mybir.dt.bfloat16,
                              name="xg", tag="xg")
                xg_tiles[R] = xg
                nc.sync.dma_start(xg[:, :], x_d[R])

            load_round(0)
            load_round(1)

            yt_pair = None
            for R in range(NROUNDS):
                if R + 2 < NROUNDS:
                    load_round(R + 2)
                xg = xg_tiles.pop(R)
                ps_tiles = [psp.tile([128, CW], mybir.dt.float32,
                                     name="ps", tag="ps")
                            for _ in range(2)]

                for j in range(KW):
                    for r2 in range(2):
                        for c2 in range(2):
                            off = c2 * SW + j
                            nc.tensor.matmul(
                                ps_tiles[r2][64 * c2:64 * c2 + SOUT, :],
                                w_sb[64 * r2:64 * r2 + 64,
                                     SOUT * j:SOUT * j + SOUT],
                                xg[64 * r2:64 * r2 + 64, off:off + CW],
                                start=(j == 0),
                                stop=(j == KW - 1),
                                tile_position=(64 * r2, 64 * c2),
                            )

                rq = R % 4
                if rq == 0:
                    yt_pair = ygp.tile([128, 8 * CW], mybir.dt.bfloat16,
                                       name="yg", tag="yg")
                for r2 in range(2):
                    dst = yt_pair[:, (2 * rq + r2) * CW:(2 * rq + r2 + 1) * CW]
                    src = ps_tiles[r2][:, :]
                    # drains alternate engines by round (stores own gpsimd,
                    # loads own sync; scalar/vector split the drain load)
                    if R % 2 == 0:
                        nc.scalar.activation(
                            dst, src, mybir.ActivationFunctionType.Copy,
                            bias=float(bias_val),
                        )
                    else:
                        nc.vector.tensor_scalar_add(dst, src, float(bias_val))

                if rq == 3 or R == NROUNDS - 1:
                    blocks = 2 * (rq + 1)
                    off = 2 * (R - rq) * CW
                    for c2 in range(2):
                        nc.gpsimd.dma_start(
                            y_d[c2, :, off:off + blocks * CW],
                            yt_pair[64 * c2:64 * c2 + SOUT, 0:blocks * CW],
                        )

    nc.compile()
    nc.finalize()
    return nc


def _toeplitz(weight: np.ndarray) -> np.ndarray:
    """[128, 7*58] bf16: block j holds T_j[u, m] = W[u-m, j] (band 0<=u-m<7),
    u in [0,64), m in [0,58); replicated for partition half r2=1."""
    t = np.zeros((TS, KW * SOUT), np.float32)
    for j in range(KW):
        for i in range(KH):
            mm = np.arange(0, SOUT)
            t[mm + i, j * SOUT + mm] = weight[i, j]
    return np.tile(t, (2, 1)).astype(_BF16)


def _pack_shard(x_bf: np.ndarray, c0: int) -> np.ndarray:
    """[18, 128, 1040] bf16: partition 64*r2 + p of round R, free slot c2
    holds row 58*(4R + 2*r2 + c2) + p."""
    valid = min(SW, W - c0)
    xs = np.zeros((PAD_ROWS, SW), _BF16)
    xs[:H, :valid] = x_bf[:, c0:c0 + valid]
    R = np.arange(NROUNDS)
    out = np.zeros((NROUNDS, 128, FREEW), _BF16)
    for r2 in range(2):
        for c2 in range(2):
            s = 4 * R + 2 * r2 + c2
            rows = SOUT * s[:, None] + np.arange(TS)[None, :]
            out[:, 64 * r2:64 * r2 + TS, c2 * SW:(c2 + 1) * SW] = xs[rows]
    return out


def _unpack_out(y_packed: np.ndarray) -> np.ndarray:
    """[2, 58, 36*512] bf16 -> [4090, 512] f32 (u = 2R+r2, strip 4R+2r2+c2)."""
    y = y_packed.reshape(2, SOUT, NROUNDS, 2, CW)   # [c2, q, R, r2, w]
    y = y.transpose(2, 3, 0, 1, 4)                  # [R, r2, c2, q, w]
    return y.reshape(NSTRIPS * SOUT, CW)[:OH].astype(np.float32)


def kernel(x: np.ndarray, weight: np.ndarray, bias: np.ndarray) -> np.ndarray:
    x = np.asarray(x, dtype=np.float32)
    weight = np.asarray(weight, dtype=np.float32)
    bias = np.asarray(bias, dtype=np.float32)

    tmat = _toeplitz(weight)
    x_bf = x.astype(_BF16)

    in_maps = []
    for c in range(NCORES):
        in_maps.append({"xs": _pack_shard(x_bf, CW * c), "tmat": tmat})

    nc = _build_program(float(bias[0]))

    trace = bool(int(os.environ.get("CONV_KERNEL_TRACE", "0")))
    res = run_bass_kernel_spmd(nc, in_maps, core_ids=list(range(NCORES)),
                               trace=trace)
    if trace:
        kernel.last_exec_time_ns = res.exec_time_ns

    cols = []
    for c in range(NCORES):
        valid_out = min(CW, OW - CW * c)
        cols.append(_unpack_out(np.asarray(res.results[c]["y"]))[:, :valid_out])
    return np.concatenate(cols, axis=1).astype(np.float32)


# revision 22
# speedup vs baseline: 1.7365x; 1.1829x over previous
"""Trainium2 kernel for 4096x4096 single-channel 7x7 valid cross-correlation + bias.

Strategy (v10): 4x concurrent 64x64 PE tiles, wide DMAs, PE-bound pipeline
--------------------------------------------------------------------------
HW model (probe-measured on this part):
- Tiled matmul throughput is bound by the serialized per-matmul LDWEIGHTS
  stream + semaphore update: ~(stationary_cols/1.2GHz + ~12ns + sem) per
  matmul; 64-col loads measure ~60-64 ns/MM.  Output rows per LDW-time is
  maximized at M=58 (64x64 tiles): 0.97 rows/ns vs 0.65 at M=26.
- DMA queues run ~3x faster on 128-partition-wide transfers than 32-wide
  ones, so the layout avoids on-chip shifted copies entirely (they would
  need narrow partition-interleaved writes).

Decomposition: y[r,c] = sum_j sum_i W[i,j] x[r+i, c+j] as 7 banded-Toeplitz
matmuls (one per kernel column j) accumulated in PSUM:
- Strips of 64 input rows -> 58 output rows; contraction K=64 (strip rows),
  lhsT [64, 58] = T_j (band W[u-m, j]); moving = strip block shifted j cols
  in the free dim (a slice, not a copy); N=512 output cols.
- 4 concurrent 64x64 PE tiles (r2, c2): tile streams SBUF partitions 64*r2,
  accumulates into PSUM bank r2 partitions [64*c2, 64*c2+58); strip
  4R + 2*r2 + c2 lives in partition half r2, free slot c2.
- Round = 4 strips: one full-width input DMA [128, 2080B lines] (sync),
  28 matmuls, 2 drains [128, 512] f32->bf16+bias (ScalarE/VectorE), and per
  round-pair 2 stores [58, 4096B lines] (gpsimd).  72 strip slots = 18
  rounds; PSUM 2 banks/round, 4-deep double buffering.

Sharding: output columns across 8 cores (512 each + 6-col halo host-side).
"""

import os

import numpy as np
import ml_dtypes

import concourse.bass as bass
import concourse.bacc as bacc_mod
import concourse.mybir as mybir
import concourse.tile as tile
from concourse.bass_utils import run_bass_kernel_spmd

H = 4096          # input rows
W = 4096          # input cols
KH = 7            # kernel rows
KW = 7            # kernel cols
OH = H - KH + 1   # 4090 output rows
OW = W - KW + 1   # 4090 output cols
NCORES = 8
CW = 512          # output cols per core
SW = CW + KW - 1  # 518 input cols per shard

TS = 64           # input rows per strip
SOUT = TS - KH + 1  # 58 output rows per strip
NROUNDS = 18      # 4 strips per round
NSTRIPS = 4 * NROUNDS           # 72 strip slots (71 real)
FREEW = 1040                    # 2 slots x 518 + pad (2080B lines)
PAD_ROWS = SOUT * (NSTRIPS - 1) + TS  # 4182

_BF16 = ml_dtypes.bfloat16


def _build_program(bias_val: float) -> bass.Bass:
    nc = bacc_mod.Bacc("TRN2", target_bir_lowering=False)

    x_d = nc.dram_tensor("xs", [NROUNDS, 128, FREEW], mybir.dt.bfloat16,
                         kind="ExternalInput")
    w_d = nc.dram_tensor("tmat", [128, KW * SOUT], mybir.dt.bfloat16,
                         kind="ExternalInput")
    # y[c2, q, u*512 + w] with u = 2R + r2: strip 4R + 2r2 + c2, row q.
    # Lines stay flat/contiguous and stores pack 4 rounds -> 16KB
    # descriptors (store queues pace ~42ns/packet regardless of size).
    y_d = nc.dram_tensor("y", [2, SOUT, 2 * NROUNDS * CW],
                         mybir.dt.bfloat16, kind="ExternalOutput")

    with tile.TileContext(nc) as tc:
        with (
            tc.tile_pool(name="const", bufs=1) as constp,
            tc.tile_pool(name="xg", bufs=NROUNDS) as xgp,
            tc.tile_pool(name="yg", bufs=5) as ygp,
            tc.tile_pool(name="ps", bufs=8, space="PSUM") as psp,
        ):
            w_sb = constp.tile([128, KW * SOUT], mybir.dt.bfloat16)
            nc.sync.dma_start(w_sb[:, :], w_d[:, :])

            xg_tiles = {}

            def load_round(R):
                xg = xgp.tile([128, FREEW], mybir.dt.bfloat16,
                              name="xg", tag="xg")
                xg_tiles[R] = xg
                nc.sync.dma_start(xg[:, :], x_d[R])

            load_round(0)
            load_round(1)

            yt_pair = None
            for R in range(NROUNDS):
                if R + 2 < NROUNDS:
                    load_round(R + 2)
                xg = xg_tiles.pop(R)
                ps_tiles = [psp.tile([128, CW], mybir.dt.float32,
                                     name="ps", tag="ps")
                            for _ in range(2)]

                for j in range(KW):
                    for r2 in range(2):
                        for c2 in range(2):
                            off = c2 * SW + j
                            nc.tensor.matmul(
                                ps_tiles[r2][64 * c2:64 * c2 + SOUT, :],
                                w_sb[64 * r2:64 * r2 + 64,
                                     SOUT * j:SOUT * j + SOUT],
                                xg[64 * r2:64 * r2 + 64, off:off + CW],
                                start=(j == 0),
                                stop=(j == KW - 1),
                                tile_position=(64 * r2, 64 * c2),
                            )

                rq = R % 4
                if rq == 0:
                    yt_pair = ygp.tile([128, 8 * CW], mybir.dt.bfloat16,
                                       name="yg", tag="yg")
                for r2 in range(2):
                    dst = yt_pair[:, (2 * rq + r2) * CW:(2 * rq + r2 + 1) * CW]
                    src = ps_tiles[r2][:, :]
                    # drains alternate engines by round (stores own gpsimd,
                    # loads own sync; scalar/vector split the drain load)
                    if R % 2 == 0:
                        nc.scalar.activation(
                            dst, src, mybir.ActivationFunctionType.Copy,
                            bias=float(bias_val),
                        )
                    else:
                        nc.vector.tensor_scalar_add(dst, src, float(bias_val))

                if rq == 3 or R == NROUNDS - 1:
                    blocks = 2 * (rq + 1)
                    off = 2 * (R - rq) * CW
                    for c2 in range(2):
                        nc.gpsimd.dma_start(
                            y_d[c2, :, off:off + blocks * CW],
                            yt_pair[64 * c2:64 * c2 + SOUT, 0:blocks * CW],
                        )

    nc.compile()
    nc.finalize()
    return nc


def _toeplitz(weight: np.ndarray) -> np.ndarray:
    """[128, 7*58] bf16: block j holds T_j[u, m] = W[u-m, j] (band 0<=u-m<7),
    u in [0,64), m in [0,58); replicated for partition half r2=1."""
    t = np.zeros((TS, KW * SOUT), np.float32)
    for j in range(KW):
        for i in range(KH):
            mm = np.arange(0, SOUT)
            t[mm + i, j * SOUT + mm] = weight[i, j]
    return np.tile(t, (2, 1)).astype(_BF16)


def _pack_shard(x_bf: np.ndarray, c0: int) -> np.ndarray:
    """[18, 128, 1040] bf16: partition 64*r2 + p of round R, free slot c2
    holds row 58*(4R + 2*r2 + c2) + p."""
    valid = min(SW, W - c0)
    xs = np.zeros((PAD_ROWS, SW), _BF16)
    xs[:H, :valid] = x_bf[:, c0:c0 + valid]
    R = np.arange(NROUNDS)
    out = np.zeros((NROUNDS, 128, FREEW), _BF16)
    for r2 in range(2):
        for c2 in range(2):
            s = 4 * R + 2 * r2 + c2
            rows = SOUT * s[:, None] + np.arange(TS)[None, :]
            out[:, 64 * r2:64 * r2 + TS, c2 * SW:(c2 + 1) * SW] = xs[rows]
    return out


def _unpack_out(y_packed: np.ndarray) -> np.ndarray:
    """[2, 58, 36*512] bf16 -> [4090, 512] f32 (u = 2R+r2, strip 4R+2r2+c2)."""
    y = y_packed.reshape(2, SOUT, NROUNDS, 2, CW)   # [c2, q, R, r2, w]
    y = y.transpose(2, 3, 0, 1, 4)                  # [R, r2, c2, q, w]
    return y.reshape(NSTRIPS * SOUT, CW)[:OH].astype(np.float32)


def kernel(x: np.ndarray, weight: np.ndarray, bias: np.ndarray) -> np.ndarray:
    x = np.asarray(x, dtype=np.float32)
    weight = np.asarray(weight, dtype=np.float32)
    bias = np.asarray(bias, dtype=np.float32)

    tmat = _toeplitz(weight)
    x_bf = x.astype(_BF16)

    in_maps = []
    for c in range(NCORES):
        in_maps.append({"xs": _pack_shard(x_bf, CW * c), "tmat": tmat})

    nc = _build_program(float(bias[0]))

    trace = bool(int(os.environ.get("CONV_KERNEL_TRACE", "0")))
    res = run_bass_kernel_spmd(nc, in_maps, core_ids=list(range(NCORES)),
                               trace=trace)
    if trace:
        kernel.last_exec_time_ns = res.exec_time_ns

    cols = []
    for c in range(NCORES):
        valid_out = min(CW, OW - CW * c)
        cols.append(_unpack_out(np.asarray(res.results[c]["y"]))[:, :valid_out])
    return np.concatenate(cols, axis=1).astype(np.float32)
